# revision 1
# baseline (speedup 1.0000x reference)
"""DCNv3 kernel for 8 Trainium2 NeuronCores.

Sharding: data-parallel over (N=4 images) x (H split in 2 halves of 32 rows)
= 8 fully independent shards (2-row x halo, no collectives).

Per-core pipeline (layouts chosen so no on-device transposes of big tensors
are needed; host pre-transposes x):
  1. x_proj = x @ W_in            (PE, fp32r; output channels sigma-permuted)
  2. z = depthwise3x3+BN(x)       (PE via 9 diagonal-block matmuls), SiLU (ACT)
  3. offsets/mask logits = x1 @ [W_off|W_mask]  (PE, (l-part, 216) layout)
  4. softmax over P + bilinear "hat" tap weights -> 5x5 dynamic-conv weight
     field W25[l, g, 5, 5]        (DVE/ACT, consolidated big-free-dim ops)
  5. W25 -> (g*25, l) via PE transposes -> DRAM; DMA replication-read
     broadcasts each group's weights across its 32 channels (bf16)
  6. apply: out[c,l] = sum_d W25[g(c),d,l] * img[c, l+shift(d)]  (DVE bf16)
  7. y = out @ W_out + b_out      (PE, fp32r) -> (256, 2048) per core
"""

import numpy as np

N, H, W, C = 4, 64, 64, 256
G, K, GC, P = 8, 3, 32, 9
BN_EPS = 1e-3
R36, CW = 36, 68          # padded shard rows / padded row width
LF = R36 * CW             # 2448
LO = 2048                 # output pixels per core (32 rows * 64)
NT = 16                   # l-tiles of 128

# sigma channel permutation: new position p holds old channel (p%8)*32 + p//8
PERM = np.array([(p % 8) * 32 + p // 8 for p in range(C)], dtype=np.int64)

_BUILT = None


def _build_bass():
    import concourse.bass as bass
    import concourse.bacc as bacc
    import concourse.mybir as mybir
    from concourse.tile import TileContext

    dt = mybir.dt
    f32, bf16, f32r = dt.float32, dt.bfloat16, dt.float32r
    AF = mybir.ActivationFunctionType
    OP = mybir.AluOpType
    AX = mybir.AxisListType

    nc = bacc.Bacc(None, target_bir_lowering=False)

    xTP_d = nc.dram_tensor("xtp", (2, 128, R36, CW), bf16, kind="ExternalInput")
    pmask_d = nc.dram_tensor("pmask", (128, R36, CW), bf16, kind="ExternalInput")
    w_in_d = nc.dram_tensor("w_in", (2, 128, 256), bf16, kind="ExternalInput")
    dwdiag_d = nc.dram_tensor("dwdiag", (2, 128, 9, 128), bf16, kind="ExternalInput")
    dwbias_d = nc.dram_tensor("dwbias", (2, 128, 1), f32, kind="ExternalInput")
    b_in_d = nc.dram_tensor("b_in", (2, 128, 1), f32, kind="ExternalInput")
    w_om_d = nc.dram_tensor("w_om", (2, 128, 256), bf16, kind="ExternalInput")
    b_om_d = nc.dram_tensor("b_om", (128, 216), f32, kind="ExternalInput")
    w_out_d = nc.dram_tensor("w_out", (2, 128, 256), bf16, kind="ExternalInput")
    b_out_d = nc.dram_tensor("b_out", (2, 128, 1), f32, kind="ExternalInput")
    identb_d = nc.dram_tensor("identb", (128, 128), bf16, kind="ExternalInput")
    wimg_d = nc.dram_tensor("wimg", (200, NT, 128), bf16, kind="Internal")
    y_d = nc.dram_tensor("yt", (2, 128, LO), f32, kind="ExternalOutput")

    with TileContext(nc) as tc:
        with (
            tc.tile_pool(name="const", bufs=1) as pc,
            tc.tile_pool(name="big", bufs=1) as pb,
            tc.tile_pool(name="work", bufs=1) as pw,
            tc.tile_pool(name="vp", bufs=1) as pv,
            tc.tile_pool(name="wbc", bufs=3) as pwb,
            tc.tile_pool(name="prod", bufs=2) as ppr,
            tc.tile_pool(name="part", bufs=1) as ppt,
            tc.tile_pool(name="psum", bufs=4, space="PSUM") as pp,
            tc.tile_pool(name="psumT", bufs=2, space="PSUM") as ppT,
        ):
            # ---------- constants ----------
            w_in = [pc.tile([128, 256], bf16, tag=f"w_in{h}", name=f"w_in{h}") for h in range(2)]
            w_om = [pc.tile([128, 256], bf16, tag=f"w_om{h}", name=f"w_om{h}") for h in range(2)]
            w_out = [pc.tile([128, 256], bf16, tag=f"w_out{h}", name=f"w_out{h}") for h in range(2)]
            dwdiag = [pb.tile([128, 9, 128], bf16, tag=f"dwd{h}", name=f"dwd{h}") for h in range(2)]
            dwbias = [pc.tile([128, 1], f32, tag=f"dwb{h}", name=f"dwb{h}") for h in range(2)]
            b_in = [pc.tile([128, 1], f32, tag=f"bin{h}", name=f"bin{h}") for h in range(2)]
            b_out = [pc.tile([128, 1], f32, tag=f"bout{h}", name=f"bout{h}") for h in range(2)]
            b_om = pc.tile([128, 216], f32, tag="b_om", name="b_om")
            identb = pc.tile([128, 128], bf16, tag="identb", name="identb")
            pmask = pc.tile([128, R36, CW], bf16, tag="pmask", name="pmask")
            for h in range(2):
                nc.sync.dma_start(w_in[h][:], w_in_d[h])
                nc.sync.dma_start(w_om[h][:], w_om_d[h])
                nc.sync.dma_start(w_out[h][:], w_out_d[h])
                nc.sync.dma_start(dwdiag[h][:], dwdiag_d[h])
                nc.sync.dma_start(dwbias[h][:], dwbias_d[h])
                nc.sync.dma_start(b_in[h][:], b_in_d[h])
                nc.sync.dma_start(b_out[h][:], b_out_d[h])
            nc.sync.dma_start(b_om[:], b_om_d[:])
            nc.sync.dma_start(identb[:], identb_d[:])
            nc.sync.dma_start(pmask[:], pmask_d[:])

            xTP = [pb.tile([128, R36, CW], bf16, tag=f"xTP{h}", name=f"xTP{h}") for h in range(2)]
            for h in range(2):
                nc.sync.dma_start(xTP[h][:], xTP_d[h])

            # ---------- 1. x_proj -> imgB0/imgB1 (bf16, bias+mask fused) --
            imgB0 = [pb.tile([128, R36, CW], bf16, tag=f"iB0{m}", name=f"iB0{m}") for m in range(2)]
            imgB1 = [pb.tile([128, R36, CW], bf16, tag=f"iB1{m}", name=f"iB1{m}") for m in range(2)]
            for m in range(2):
                for ch in range(6):
                    ps = pp.tile([128, 512], f32, tag="ps", name="ps")
                    for kh in range(2):
                        nc.tensor.matmul(
                            ps[:, 0:408],
                            w_in[kh][:, m * 128:(m + 1) * 128],
                            xTP[kh][:, ch * 6:(ch + 1) * 6, :],
                            start=(kh == 0), stop=(kh == 1),
                        )
                    nc.vector.scalar_tensor_tensor(
                        imgB0[m][:, ch * 6:(ch + 1) * 6, :]
                            .rearrange("p r c -> p (r c)"),
                        ps[:, 0:408], b_in[m][:, 0:1],
                        pmask[:, ch * 6:(ch + 1) * 6, :]
                            .rearrange("p r c -> p (r c)"),
                        op0=OP.add, op1=OP.mult,
                    )
            for m in range(2):
                nc.vector.tensor_copy(
                    imgB1[m][:, :, 0:CW - 1], imgB0[m][:, :, 1:CW])

            # ---------- 2. depthwise conv + BN + SiLU -> x1T ----------
            x1T = [pb.tile([128, 32, 64], bf16, tag=f"x1T{h}", name=f"x1T{h}") for h in range(2)]
            for hf in range(2):
                for ch in range(4):
                    ps = pp.tile([128, 512], f32, tag="ps", name="ps")
                    r0 = 2 + ch * 8
                    for d in range(9):
                        ky, kx = d // 3, d % 3
                        nc.tensor.matmul(
                            ps[:],
                            dwdiag[hf][:, d, :],
                            xTP[hf][:, r0 + ky - 1:r0 + ky + 7,
                                    1 + kx:65 + kx],
                            start=(d == 0), stop=(d == 8),
                        )
                    nc.scalar.activation(
                        x1T[hf][:, ch * 8:(ch + 1) * 8, :]
                            .rearrange("p r c -> p (r c)"),
                        ps[:], AF.Silu, bias=dwbias[hf][:, 0:1], scale=1.0,
                    )

            # ---------- 3. offset/mask head ----------
            om = pb.tile([128, NT, 8, 27], f32, tag="om", name="om")
            for t in range(NT):
                ps = pp.tile([128, 256], f32, tag="ps", name="ps")
                for kh in range(2):
                    nc.tensor.matmul(
                        ps[:],
                        x1T[kh][:].rearrange("p r c -> p (r c)")
                            [:, t * 128:(t + 1) * 128],
                        w_om[kh][:],
                        start=(kh == 0), stop=(kh == 1),
                    )
                nc.vector.tensor_tensor(
                    om[:, t].rearrange("p g j -> p (g j)"),
                    ps[:, 0:216], b_om[:], op=OP.add,
                )

            # ---------- 4. tap-weight field W25 ----------
            omi = om[:, :, :, 0:18].rearrange(
                "p t g (n two) -> p t g n two", two=2)
            ox_v = omi[:, :, :, :, 0]      # (128, NT, 8, 9)
            oy_v = omi[:, :, :, :, 1]
            lg_v = om[:, :, :, 18:27]

            E = pw.tile([128, NT, 8, 9], f32, tag="E", name="E")
            nc.scalar.activation(E[:], lg_v, AF.Exp)
            S = pw.tile([128, NT, 8, 1], f32, tag="S", name="S")
            nc.vector.tensor_reduce(S[:], E[:], axis=AX.X, op=OP.add)
            R = pw.tile([128, NT, 8, 1], f32, tag="R", name="R")
            nc.vector.reciprocal(R[:], S[:])
            msk = pw.tile([128, NT, 8, 9], f32, tag="msk", name="msk")
            nc.vector.tensor_tensor(
                msk[:], E[:], R[:].to_broadcast([128, NT, 8, 9]), op=OP.mult)

            def hats(o_view, tag):
                h3 = pw.tile([128, NT, 8, 9, 3], f32, tag=tag)
                tmp = pw.tile([128, NT, 8, 9], f32, tag=tag + "_t")
                nc.vector.tensor_scalar(
                    h3[:, :, :, :, 0], o_view, -1.0, 0.0,
                    op0=OP.mult, op1=OP.max)
                nc.vector.tensor_scalar(
                    h3[:, :, :, :, 2], o_view, 0.0, None, op0=OP.max)
                nc.vector.tensor_tensor(
                    tmp[:], h3[:, :, :, :, 0], h3[:, :, :, :, 2], op=OP.add)
                nc.vector.tensor_scalar(
                    h3[:, :, :, :, 1], tmp[:], -1.0, 1.0,
                    op0=OP.mult, op1=OP.add)
                return h3

            hy3 = hats(oy_v, "hy3")
            hx3 = hats(ox_v, "hx3")
            nc.vector.tensor_tensor(
                hy3[:], hy3[:],
                msk[:].to_broadcast([128, NT, 8, 9, 3]),
                op=OP.mult)

            W25 = pb.tile([128, NT, 8, 5, 5], f32, tag="W25", name="W25")
            nc.vector.memset(W25[:], 0.0)
            for p in range(P):
                kw, ky = p // 3 - 1, p % 3 - 1  # torch: p = iw*K + ih
                Vp = pv.tile([128, NT, 8, 3, 3], f32, tag="Vp", name="Vp")
                nc.vector.tensor_tensor(
                    Vp[:],
                    hy3[:, :, :, p, :].to_broadcast([128, NT, 8, 3, 3]),
                    hx3[:, :, :, p, :].rearrange("p t g (a b) -> p t g a b",
                                                 a=1)
                        .to_broadcast([128, NT, 8, 3, 3]),
                    op=OP.mult)
                wv = W25[:, :, :, ky + 1:ky + 4, kw + 1:kw + 4]
                nc.vector.tensor_tensor(wv, wv, Vp[:], op=OP.add)

            # ---------- 5. W25 -> (g*25, l) bf16 -> DRAM ----------
            W25b = pb.tile([128, NT, 200], bf16, tag="xTP0", name="W25b")
            nc.vector.tensor_copy(
                W25b[:], W25[:].rearrange("p t g a b -> p t (g a b)"))
            WTa = pb.tile([128, NT, 128], bf16, tag="WTa", name="WTa")
            WTb = pb.tile([72, NT, 128], bf16, tag="WTb", name="WTb")
            for t in range(NT):
                psA = ppT.tile([128, 128], bf16, tag="psT", name="psT")
                nc.tensor.transpose(psA[:], W25b[:, t, 0:128], identb[:])
                nc.vector.tensor_copy(WTa[:, t, :], psA[:])
                psB = ppT.tile([72, 128], bf16, tag="psT", name="psT")
                nc.tensor.transpose(psB[:], W25b[:, t, 128:200], identb[:])
                nc.vector.tensor_copy(WTb[:, t, :], psB[:])
            nc.sync.dma_start(wimg_d[0:128], WTa[:])
            nc.sync.dma_start(wimg_d[128:200], WTb[:])

            # ---------- 6. apply ----------
            acc = [pb.tile([128, LO], bf16, tag=f"dwd{h}", name=f"acc{h}") for h in range(2)]
            wimg_g = wimg_d[:].rearrange("(g d) t l -> g d (t l)", d=25)
            for hf in range(2):
                partials = []
                for dy in range(-2, 3):
                    pp_t = ppt.tile([128, LO], bf16, tag=f"pp{dy}", name=f"pp{dy}")
                    for dx in range(-2, 3):
                        d = (dy + 2) * 5 + (dx + 2)
                        wb = pwb.tile([128, LO], bf16, tag="wb", name="wb")
                        nc.sync.dma_start(
                            wb[:],
                            wimg_g[:, d, :]
                                .rearrange("(a g) l -> a g l", a=1)
                                .to_broadcast([16, 8, NT * 128]))
                        img = imgB0 if dx % 2 == 0 else imgB1
                        cb = (2 + dx) - (dx % 2)
                        iv = img[hf][:, 2 + dy:34 + dy, cb:cb + 64]
                        pr = ppr.tile([128, 32, 64], bf16, tag="pr", name="pr")
                        nc.vector.tensor_tensor(
                            pr[:], wb[:].rearrange("p (r c) -> p r c", r=32),
                            iv, op=OP.mult)
                        prf = pr[:].rearrange("p r c -> p (r c)")
                        if dx == -2:
                            nc.vector.tensor_copy(pp_t[:], prf)
                        else:
                            nc.vector.tensor_tensor(
                                pp_t[:], pp_t[:], prf, op=OP.add)
                    partials.append(pp_t)
                nc.vector.tensor_tensor(
                    partials[0][:], partials[0][:], partials[1][:], op=OP.add)
                nc.vector.tensor_tensor(
                    partials[2][:], partials[2][:], partials[3][:], op=OP.add)
                nc.vector.tensor_tensor(
                    partials[0][:], partials[0][:], partials[2][:], op=OP.add)
                nc.vector.tensor_tensor(
                    acc[hf][:], partials[0][:], partials[4][:], op=OP.add)

            # ---------- 7. y = acc @ W_out + b_out ----------
            yT = [pb.tile([128, LO], f32, tag=f"xTP{m}", name=f"yT{m}") for m in range(2)]
            for mo in range(2):
                for ch in range(4):
                    ps = pp.tile([128, 512], f32, tag="ps", name="ps")
                    for kh in range(2):
                        nc.tensor.matmul(
                            ps[:],
                            w_out[kh][:, mo * 128:(mo + 1) * 128],
                            acc[kh][:, ch * 512:(ch + 1) * 512],
                            start=(kh == 0), stop=(kh == 1),
                        )
                    nc.vector.tensor_scalar_add(
                        yT[mo][:, ch * 512:(ch + 1) * 512], ps[:],
                        b_out[mo][:, 0:1],
                    )
                nc.sync.dma_start(y_d[mo], yT[mo][:])

    nc.finalize()
    return nc


def _host_prep(inputs):
    x = np.asarray(inputs["x"], np.float32)
    dw_w = np.asarray(inputs["dw_w"], np.float32)
    bn_gamma = np.asarray(inputs["bn_gamma"], np.float32)
    bn_beta = np.asarray(inputs["bn_beta"], np.float32)
    bn_mean = np.asarray(inputs["bn_mean"], np.float32)
    bn_var = np.asarray(inputs["bn_var"], np.float32)
    W_off = np.asarray(inputs["W_off"], np.float32)
    b_off = np.asarray(inputs["b_off"], np.float32)
    W_mask = np.asarray(inputs["W_mask"], np.float32)
    b_mask = np.asarray(inputs["b_mask"], np.float32)
    W_in = np.asarray(inputs["W_in"], np.float32)
    b_in_v = np.asarray(inputs["b_in"], np.float32)
    W_out = np.asarray(inputs["W_out"], np.float32)
    b_out_v = np.asarray(inputs["b_out"], np.float32)
    import ml_dtypes

    scale = bn_gamma / np.sqrt(bn_var + BN_EPS)
    dwS = dw_w[:, 0] * scale[:, None, None]
    dwbias = (bn_beta - bn_mean * scale).astype(np.float32)

    dwdiag = np.zeros((2, 128, 9, 128), np.float32)
    for hf in range(2):
        cs = slice(hf * 128, (hf + 1) * 128)
        for d in range(9):
            ky, kx = d // 3, d % 3
            dwdiag[hf, :, d, :] = np.diag(dwS[cs, ky, kx])

    w_om = np.zeros((C, 256), np.float32)
    b_om_v = np.zeros(216, np.float32)
    for g in range(G):
        w_om[:, g * 27:g * 27 + 18] = W_off[:, g * 18:(g + 1) * 18]
        w_om[:, g * 27 + 18:g * 27 + 27] = W_mask[:, g * 9:(g + 1) * 9]
        b_om_v[g * 27:g * 27 + 18] = b_off[g * 18:(g + 1) * 18]
        b_om_v[g * 27 + 18:g * 27 + 27] = b_mask[g * 9:(g + 1) * 9]

    common = {
        "w_in": np.ascontiguousarray(W_in[:, PERM].reshape(2, 128, 256)).astype(
            ml_dtypes.bfloat16),
        "dwdiag": dwdiag.astype(ml_dtypes.bfloat16),
        "dwbias": dwbias.reshape(2, 128, 1),
        "b_in": np.ascontiguousarray(b_in_v[PERM].reshape(2, 128, 1)),
        "w_om": np.ascontiguousarray(w_om.reshape(2, 128, 256)).astype(
            ml_dtypes.bfloat16),
        "b_om": np.tile(b_om_v, (128, 1)).astype(np.float32),
        "w_out": np.ascontiguousarray(W_out[PERM, :].reshape(2, 128, 256)).astype(
            ml_dtypes.bfloat16),
        "b_out": b_out_v.reshape(2, 128, 1).astype(np.float32),
        "identb": np.eye(128, dtype=np.float32).astype(ml_dtypes.bfloat16),
    }

    in_maps = []
    for core in range(8):
        n, half = core // 2, core % 2
        h0 = half * 32
        rows = np.zeros((R36, CW, C), np.float32)
        lo, hi = max(0, h0 - 2), min(H, h0 + 34)
        rows[(lo - (h0 - 2)):(hi - (h0 - 2)), 2:66, :] = x[n, lo:hi]
        xtp = np.ascontiguousarray(
            rows.reshape(LF, C).T.reshape(2, 128, R36, CW)).astype(
                ml_dtypes.bfloat16)
        pm = np.zeros((R36, CW), np.float32)
        pm[(lo - (h0 - 2)):(hi - (h0 - 2)), 2:66] = 1.0
        m = dict(common)
        m["xtp"] = xtp
        m["pmask"] = np.ascontiguousarray(
            np.broadcast_to(pm, (128, R36, CW))).astype(ml_dtypes.bfloat16)
        in_maps.append(m)
    return in_maps


def kernel(**inputs):
    global _BUILT
    from concourse.bass_utils import run_bass_kernel_spmd

    if _BUILT is None:
        _BUILT = _build_bass()
    nc = _BUILT

    in_maps = _host_prep(inputs)
    res = run_bass_kernel_spmd(nc, in_maps, core_ids=list(range(8)))

    out = np.zeros((N, H, W, C), np.float32)
    for core in range(8):
        n, half = core // 2, core % 2
        yt = np.asarray(res.results[core]["yt"], np.float32).reshape(256, LO)
        out[n, half * 32:(half + 1) * 32] = yt.T.reshape(32, 64, 256)
    return out


if __name__ == "__main__":
    import reference
    inputs = {k: np.asarray(v) for k, v in reference.setup_inputs().items()}
    got = kernel(**inputs)
    exp = np.asarray(reference.reference(**inputs))
    rel = np.linalg.norm(got - exp) / np.linalg.norm(exp)
    print("max abs err:", np.abs(got - exp).max(), "rel:", rel)

